# revision 9
# baseline (speedup 1.0000x reference)
"""Trainium2 Bass kernel for GQA multi-head attention with RoPE + QK-RMSNorm
and a block-staircase mask (block-diffusion attention).

Strategy: data-parallel over batch across 8 NeuronCores (2 batches/core).
Per core, fully fused pipeline in fp32r (TF32-like) matmuls:
  x -> xT (PE transpose) -> q/k/v projections -> RoPE+RMSNorm (DVE/ACT)
  -> qT/kT via PE transpose (head-paired layout, halves of 128 partitions)
  -> scores S^T = kT.T @ qT per kv-group (K=64 matmuls, both halves)
  -> exp (ACT, fused 1/sqrt(d) scale) -> 0/1 mask multiply (DVE)
  -> AV: yT = [v|1].T @ E^T accumulated over key tiles (denominator fused
     as an extra ones column -> Z row) -> ones-matmul partition broadcast
     of Z -> reciprocal -> divide -> Wo projection -> y.

The attention mask is analyzed host-side per 128x128 block (full / zero /
partial); zero blocks are skipped, partial blocks get a 0/1 multiply.
"""
import sys

sys.path.insert(0, "/opt/trn_rl_repo")

from contextlib import ExitStack

import numpy as np

import concourse.bass as bass
import concourse.tile as tile
from concourse import bacc, mybir
from concourse.bass_utils import run_bass_kernel_spmd

F32 = mybir.dt.float32
F32R = mybir.dt.float32r
ACT = mybir.ActivationFunctionType
AX = mybir.AxisListType

B, T, C = 16, 1024, 1024
H, G, D = 16, 4, 64          # query heads, kv heads, head dim
NCORES = 8
BL = B // NCORES             # batches per core
TT = T // 128                # 8 row tiles
QCN = 4                      # 256-wide query chunks per sequence
EPS = 1e-6
SCALE = 1.0 / np.sqrt(D)

# head-pair permutation: transpose pair j holds (PI[2j] -> half0, PI[2j+1] -> half1)
# so kv-group g lives in half g%2, slots (g//2)*4..+3, head = 4g + slot-offset
PI = []
for _j in range(8):
    _a, _b = (_j, _j + 4) if _j < 4 else (_j + 4, _j + 8)
    PI += [_a, _b]


def _analyze_mask(attn_mask):
    """Per 128x128 block classification in S^T orientation."""
    classes = [[None] * TT for _ in range(TT)]
    tiles = []
    for qt in range(TT):
        for kt in range(TT):
            blk = attn_mask[qt * 128:(qt + 1) * 128, kt * 128:(kt + 1) * 128]
            if blk.all():
                classes[qt][kt] = "full"
            elif not blk.any():
                classes[qt][kt] = "zero"
            else:
                mt = blk.T.astype(np.float32)  # S^T orientation: [key, query]
                for i, ex in enumerate(tiles):
                    if np.array_equal(ex, mt):
                        classes[qt][kt] = i
                        break
                else:
                    tiles.append(mt)
                    classes[qt][kt] = len(tiles) - 1
    return classes, tiles


def build_program(classes, n_masks, use_qw, use_kw):
    nc = bacc.Bacc("TRN2", target_bir_lowering=False, debug=False)

    x_d = nc.dram_tensor("x", [BL * T, C], F32, kind="ExternalInput")
    wq_d = nc.dram_tensor("wq", [C, H * D], F32, kind="ExternalInput")
    wk_d = nc.dram_tensor("wk", [C, G * D], F32, kind="ExternalInput")
    wv_d = nc.dram_tensor("wv", [C, G * D], F32, kind="ExternalInput")
    wo_d = nc.dram_tensor("wo", [H * D, C], F32, kind="ExternalInput")
    cos_d = nc.dram_tensor("cos", [T, D // 2], F32, kind="ExternalInput")
    sin_d = nc.dram_tensor("sin", [T, D // 2], F32, kind="ExternalInput")
    id_d = nc.dram_tensor("ident", [128, 128], F32, kind="ExternalInput")
    nm = max(1, n_masks)
    mask_d = nc.dram_tensor("masks", [nm, 128, 128], F32, kind="ExternalInput")
    wqn_d = nc.dram_tensor("wqn", [128, 1], F32, kind="ExternalInput")
    wkn_d = nc.dram_tensor("wkn", [128, 1], F32, kind="ExternalInput")
    y_d = nc.dram_tensor("y", [BL * T, C], F32, kind="ExternalOutput")

    with tile.TileContext(nc) as tc, ExitStack() as st:
        const = st.enter_context(tc.tile_pool(name="const", bufs=1))
        scr = st.enter_context(tc.tile_pool(name="scr", bufs=2))
        bat = st.enter_context(tc.tile_pool(name="bat", bufs=1))
        xnp = st.enter_context(tc.tile_pool(name="xnp", bufs=2))
        ropep = st.enter_context(tc.tile_pool(name="ropep", bufs=3))
        small = st.enter_context(tc.tile_pool(name="small", bufs=2))
        qtp = st.enter_context(tc.tile_pool(name="qtp", bufs=1))
        erp = st.enter_context(tc.tile_pool(name="erp", bufs=2))
        yp = st.enter_context(tc.tile_pool(name="yp", bufs=1))
        youtp = st.enter_context(tc.tile_pool(name="youtp", bufs=2))
        knp = st.enter_context(tc.tile_pool(name="knp", bufs=1))

        pp = st.enter_context(tc.tile_pool(name="pp", bufs=2, space="PSUM"))
        ptp = st.enter_context(tc.tile_pool(name="ptp", bufs=2, space="PSUM"))
        psp = st.enter_context(tc.tile_pool(name="psp", bufs=2, space="PSUM"))
        pyp = st.enter_context(tc.tile_pool(name="pyp", bufs=1, space="PSUM"))

        # ---------------- constants ----------------
        id_sb = const.tile([128, 128], F32, tag="id")
        nc.sync.dma_start(id_sb[:], id_d[:])
        cos_sb = const.tile([128, TT, 32], F32, tag="cos")
        sin_sb = const.tile([128, TT, 32], F32, tag="sin")
        nc.sync.dma_start(cos_sb[:], cos_d.ap().rearrange("(tt p) d -> p tt d", p=128))
        nc.sync.dma_start(sin_sb[:], sin_d.ap().rearrange("(tt p) d -> p tt d", p=128))

        maskr = const.tile([128, nm, 128], F32R, tag="maskr")
        mf = scr.tile([128, nm, 128], F32, tag="wchunk")
        nc.sync.dma_start(mf[:], mask_d.ap().rearrange("n p k -> p n k"))
        nc.scalar.activation(maskr[:], mf[:], ACT.Copy)

        onesr = const.tile([128, 64], F32R, tag="onesr")
        ones_f = scr.tile([128, 64], F32, tag="wchunk")
        nc.vector.memset(ones_f[:], 1.0)
        nc.scalar.activation(onesr[64:65, :], ones_f[64:65, :], ACT.Copy)

        w2q = const.tile([128, 1], F32, tag="w2q")
        w2k = const.tile([128, 1], F32, tag="w2k")
        nc.sync.dma_start(w2q[:], wqn_d[:])
        nc.sync.dma_start(w2k[:], wkn_d[:])
        epsb = const.tile([128, 1], F32, tag="epsb")
        nc.vector.memset(epsb[:], EPS)

        # weights, rounded to f32r through a staging chunk
        wqr = const.tile([128, 8, H * D], F32R, tag="wqr")
        wkr = const.tile([128, 8, G * D], F32R, tag="wkr")
        wvr = const.tile([128, 8, G * D], F32R, tag="wvr")
        wor = const.tile([128, 8, C], F32R, tag="wor")
        for w_d, w_r, m in ((wq_d, wqr, H * D), (wk_d, wkr, G * D),
                            (wv_d, wvr, G * D), (wo_d, wor, C)):
            wv_ap = w_d.ap().rearrange("(ct p) m -> p ct m", p=128)
            for ct in range(8):
                chunk = scr.tile([128, 1024], F32, tag="wchunk")
                nc.sync.dma_start(chunk[:, 0:m], wv_ap[:, ct, :])
                nc.scalar.activation(w_r[:, ct, :], chunk[:, 0:m], ACT.Copy)

        # ---------------- per batch ----------------
        for b in range(BL):
            xTr = bat.tile([128, 8, T], F32R, tag="xTr")     # [c_part, ct, t]
            kTr = bat.tile([128, 2, T], F32R, tag="kTr")     # [d(2 kv), pair, t]
            vr = bat.tile([128, TT, G, 65], F32R, tag="vr")  # [key, tt, g, d+1]

            # ones column of v_ext
            of = scr.tile([128, 32], F32, tag="wchunk")
            nc.vector.memset(of[:], 1.0)
            nc.vector.tensor_copy(
                vr[:, :, :, 64:65],
                of[:].rearrange("p (a b c) -> p a b c", a=TT, b=G))

            # --- xT ---
            for tt in range(TT):
                xn = xnp.tile([128, C], F32, tag="xn")
                nc.sync.dma_start(
                    xn[:], x_d[b * T + tt * 128: b * T + (tt + 1) * 128, :])
                for ct in range(8):
                    ptile = ptp.tile([128, 128], F32, tag="pt")
                    nc.tensor.transpose(ptile[:], xn[:, ct * 128:(ct + 1) * 128],
                                        id_sb[:])
                    nc.scalar.activation(xTr[:, ct, tt * 128:(tt + 1) * 128],
                                         ptile[:], ACT.Copy)

            # --- k/v projections + rope/norm + transpose ---
            for tt in range(TT):
                pk = pp.tile([128, 512], F32, tag="pp")
                for ct in range(8):
                    nc.tensor.matmul(pk[:, 0:G * D],
                                     xTr[:, ct, tt * 128:(tt + 1) * 128],
                                     wkr[:, ct, :], start=(ct == 0), stop=(ct == 7))
                pk4 = pk[:, 0:G * D].rearrange("p (g h d) -> p g h d", g=G, h=2)
                cosb = cos_sb[:, tt, :].unsqueeze(1).unsqueeze(1).to_broadcast(
                    (128, G, 2, 32))
                sinb = sin_sb[:, tt, :].unsqueeze(1).unsqueeze(1).to_broadcast(
                    (128, G, 2, 32))
                ka = ropep.tile([128, 1024], F32, tag="rope")
                kb = ropep.tile([128, 1024], F32, tag="rope")
                ka4 = ka[:, 0:G * D].rearrange("p (g h d) -> p g h d", g=G, h=2)
                kb4 = kb[:, 0:G * D].rearrange("p (g h d) -> p g h d", g=G, h=2)
                nc.vector.tensor_mul(ka4, pk4, cosb)
                nc.vector.tensor_mul(kb4, pk4, sinb)
                kn = knp.tile([128, G * D], F32, tag="knorm")
                kn4 = kn[:].rearrange("p (g h d) -> p g h d", g=G, h=2)
                nc.vector.tensor_add(kn4[:, :, 0, :], ka4[:, :, 0, :], kb4[:, :, 1, :])
                nc.vector.tensor_sub(kn4[:, :, 1, :], ka4[:, :, 1, :], kb4[:, :, 0, :])
                sq = ropep.tile([128, 1024], F32, tag="rope")
                nc.vector.tensor_mul(sq[:, 0:G * D], kn[:], kn[:])
                ss = small.tile([128, G], F32, tag="ss")
                nc.vector.reduce_sum(
                    ss[:], sq[:, 0:G * D].rearrange("p (g d) -> p g d", g=G),
                    axis=AX.X)
                srt = small.tile([128, G], F32, tag="srt")
                nc.scalar.activation(srt[:], ss[:], ACT.Sqrt, bias=epsb[:], scale=1.0 / D)
                sinv = small.tile([128, G], F32, tag="sinv")
                nc.vector.reciprocal(sinv[:], srt[:])
                nc.vector.tensor_mul(
                    kn[:].rearrange("p (g d) -> p g d", g=G),
                    kn[:].rearrange("p (g d) -> p g d", g=G),
                    sinv[:].unsqueeze(2).to_broadcast((128, G, D)))
                for j in range(2):
                    ptile = ptp.tile([128, 128], F32, tag="pt")
                    nc.tensor.transpose(ptile[:], kn[:, j * 128:(j + 1) * 128],
                                        id_sb[:])
                    kw = {"scale": w2k[:]} if use_kw else {}
                    nc.scalar.activation(kTr[:, j, tt * 128:(tt + 1) * 128],
                                         ptile[:], ACT.Copy, **kw)
                pv = pp.tile([128, 512], F32, tag="pp")
                for ct in range(8):
                    nc.tensor.matmul(pv[:, 0:G * D],
                                     xTr[:, ct, tt * 128:(tt + 1) * 128],
                                     wvr[:, ct, :], start=(ct == 0), stop=(ct == 7))
                nc.scalar.activation(
                    vr[:, tt, :, 0:64],
                    pv[:, 0:G * D].rearrange("p (g d) -> p g d", g=G), ACT.Copy)

            # --- query chunks ---
            for qc in range(QCN):
                qTr = qtp.tile([128, 8, 256], F32R, tag="qTr")  # [d, slot, tq]
                for tl in range(2):
                    tt = 2 * qc + tl
                    qn = ropep.tile([128, 1024], F32, tag="qn", bufs=2)
                    for mh in range(2):
                        pq = pp.tile([128, 512], F32, tag="pp")
                        for ct in range(8):
                            nc.tensor.matmul(
                                pq[:], xTr[:, ct, tt * 128:(tt + 1) * 128],
                                wqr[:, ct, mh * 512:(mh + 1) * 512],
                                start=(ct == 0), stop=(ct == 7))
                        pq4 = pq[:].rearrange("p (g h d) -> p g h d", g=8, h=2)
                        cosb = cos_sb[:, tt, :].unsqueeze(1).unsqueeze(1).to_broadcast(
                            (128, 8, 2, 32))
                        sinb = sin_sb[:, tt, :].unsqueeze(1).unsqueeze(1).to_broadcast(
                            (128, 8, 2, 32))
                        qa = ropep.tile([128, 1024], F32, tag="rope")
                        qb = ropep.tile([128, 1024], F32, tag="rope")
                        qa4 = qa[:, 0:512].rearrange("p (g h d) -> p g h d", g=8, h=2)
                        qb4 = qb[:, 0:512].rearrange("p (g h d) -> p g h d", g=8, h=2)
                        nc.vector.tensor_mul(qa4, pq4, cosb)
                        nc.vector.tensor_mul(qb4, pq4, sinb)
                        qn4 = qn[:, mh * 512:(mh + 1) * 512].rearrange(
                            "p (g h d) -> p g h d", g=8, h=2)
                        nc.vector.tensor_add(qn4[:, :, 0, :], qa4[:, :, 0, :],
                                             qb4[:, :, 1, :])
                        nc.vector.tensor_sub(qn4[:, :, 1, :], qa4[:, :, 1, :],
                                             qb4[:, :, 0, :])
                        sq = ropep.tile([128, 1024], F32, tag="rope")
                        nc.vector.tensor_mul(sq[:, 0:512],
                                             qn[:, mh * 512:(mh + 1) * 512],
                                             qn[:, mh * 512:(mh + 1) * 512])
                        ss = small.tile([128, 8], F32, tag="ss")
                        nc.vector.reduce_sum(
                            ss[:], sq[:, 0:512].rearrange("p (g d) -> p g d", g=8),
                            axis=AX.X)
                        srt = small.tile([128, 8], F32, tag="srt")
                        nc.scalar.activation(srt[:], ss[:], ACT.Sqrt, bias=epsb[:],
                                             scale=1.0 / D)
                        sinv = small.tile([128, 8], F32, tag="sinv")
                        nc.vector.reciprocal(sinv[:], srt[:])
                        nc.vector.tensor_mul(
                            qn[:, mh * 512:(mh + 1) * 512].rearrange(
                                "p (g d) -> p g d", g=8),
                            qn[:, mh * 512:(mh + 1) * 512].rearrange(
                                "p (g d) -> p g d", g=8),
                            sinv[:].unsqueeze(2).to_broadcast((128, 8, D)))
                    for j in range(8):
                        ptile = ptp.tile([128, 128], F32, tag="pt")
                        nc.tensor.transpose(ptile[:], qn[:, j * 128:(j + 1) * 128],
                                            id_sb[:])
                        kw = {"scale": w2q[:]} if use_qw else {}
                        nc.scalar.activation(qTr[:, j, tl * 128:(tl + 1) * 128],
                                             ptile[:], ACT.Copy, **kw)

                # --- attention for this chunk ---
                yup = yp.tile([128, 8, 256], F32, tag="yup")
                yTwo = yp.tile([128, 8, 256], F32R, tag="yTwo")

                kjs = []
                for kj in range(TT):
                    cls = [classes[2 * qc + tl][kj] for tl in range(2)]
                    if cls[0] == "zero" and cls[1] == "zero":
                        continue
                    kjs.append((kj, cls))

                for g in range(G):
                    u = g % 2
                    sb0 = (g // 2) * 4
                    py = pyp.tile([65, 4, 256], F32, tag="py")
                    for idx, (kj, cls) in enumerate(kjs):
                        er = erp.tile([128, 4, 256], F32R, tag="er")
                        for sp in range(2):
                            ps = psp.tile([128, 512], F32, tag="ps")
                            nc.tensor.matmul(
                                ps[:],
                                kTr[u * 64:(u + 1) * 64, g // 2,
                                    kj * 128:(kj + 1) * 128],
                                qTr[u * 64:(u + 1) * 64,
                                    sb0 + 2 * sp: sb0 + 2 * sp + 2, :],
                                tile_position=(u * 64, 0))
                            nc.scalar.activation(
                                er[:, 2 * sp:2 * sp + 2, :].rearrange(
                                    "p a b -> p (a b)"),
                                ps[:], ACT.Exp, scale=float(SCALE))
                        for tl in range(2):
                            c = cls[tl]
                            if c == "full":
                                continue
                            reg = er[:, :, tl * 128:(tl + 1) * 128]
                            if c == "zero":
                                nc.vector.tensor_scalar_mul(reg, reg, 0.0)
                            else:
                                nc.vector.tensor_mul(
                                    reg, reg,
                                    maskr[:, c:c + 1, :].to_broadcast((128, 4, 128)))
                        last = idx == len(kjs) - 1
                        for j in range(4):
                            # j=0/2 open the two psum banks (bank-wide
                            # has_written clear); j=1/3 land on cleared bits ->
                            # overwrite; everything else accumulates.
                            nc.tensor.matmul(py[:, j, :], vr[:, kj, g, :],
                                             er[:, j, :],
                                             start=(idx == 0 and j % 2 == 0),
                                             stop=(last and j % 2 == 1))
                    # denominator: Z row -> broadcast -> reciprocal -> divide
                    zr = ropep.tile([65, 1024], F32R, tag="rope")
                    nc.scalar.activation(
                        zr[64:65, :],
                        py[64:65, :, :].rearrange("p a b -> p (a b)"), ACT.Copy)
                    zinv = ropep.tile([64, 1024], F32, tag="rope")
                    for zh in range(2):
                        pz = psp.tile([64, 512], F32, tag="ps")
                        nc.tensor.matmul(pz[:], onesr[64:65, :],
                                         zr[64:65, zh * 512:(zh + 1) * 512],
                                         tile_position=(64, 0))
                        nc.vector.reciprocal(zinv[:, zh * 512:(zh + 1) * 512], pz[:])
                    for j in range(4):
                        h = 4 * g + j
                        p = h // 2
                        if h % 2 == 0:
                            nc.vector.tensor_mul(yTwo[0:64, p, :], py[0:64, j, :],
                                                 zinv[:, j * 256:(j + 1) * 256])
                        else:
                            nc.vector.tensor_mul(yup[0:64, p, :], py[0:64, j, :],
                                                 zinv[:, j * 256:(j + 1) * 256])
                # shift odd heads to upper partitions and round to f32r
                nc.sync.dma_start(yup[64:128, :, :], yup[0:64, :, :])
                nc.scalar.activation(yTwo[64:128, :, :], yup[64:128, :, :], ACT.Copy)

                # --- Wo projection ---
                for tl in range(2):
                    for nh in range(2):
                        po = psp.tile([128, 512], F32, tag="ps")
                        for mt in range(8):
                            nc.tensor.matmul(
                                po[:], yTwo[:, mt, tl * 128:(tl + 1) * 128],
                                wor[:, mt, nh * 512:(nh + 1) * 512],
                                start=(mt == 0), stop=(mt == 7))
                        yout = youtp.tile([128, 512], F32, tag="yout")
                        nc.scalar.activation(yout[:], po[:], ACT.Copy)
                        r0 = b * T + qc * 256 + tl * 128
                        nc.sync.dma_start(
                            y_d[r0:r0 + 128, nh * 512:(nh + 1) * 512], yout[:])

    nc.compile()
    return nc


_CACHE = {}


def _prepare(attn_mask, q_norm_w, k_norm_w):
    classes, tiles = _analyze_mask(np.asarray(attn_mask, dtype=bool))
    use_qw = not np.allclose(np.asarray(q_norm_w), 1.0)
    use_kw = not np.allclose(np.asarray(k_norm_w), 1.0)
    key = (tuple(tuple(r) for r in classes), len(tiles), use_qw, use_kw)
    if key not in _CACHE:
        _CACHE[key] = (build_program(classes, len(tiles), use_qw, use_kw), tiles)
    return _CACHE[key]


def make_in_maps(x, cos, sin, attn_mask, Wq, Wk, Wv, Wo, q_norm_w, k_norm_w, tiles):
    x = np.ascontiguousarray(np.asarray(x, dtype=np.float32))
    cosf = np.ascontiguousarray(np.asarray(cos, np.float32).reshape(T, D // 2))
    sinf = np.ascontiguousarray(np.asarray(sin, np.float32).reshape(T, D // 2))

    wq = np.asarray(Wq, dtype=np.float32)
    wq_p = np.empty_like(wq)
    for j in range(8):
        a, b2 = PI[2 * j], PI[2 * j + 1]
        wq_p[:, 128 * j:128 * j + 64] = wq[:, 64 * a:64 * a + 64]
        wq_p[:, 128 * j + 64:128 * j + 128] = wq[:, 64 * b2:64 * b2 + 64]

    nmask = max(1, len(tiles))
    maskdata = np.zeros((nmask, 128, 128), dtype=np.float32)
    for i, t in enumerate(tiles):
        maskdata[i] = t

    shared = {
        "wq": np.ascontiguousarray(wq_p),
        "wk": np.ascontiguousarray(np.asarray(Wk, dtype=np.float32)),
        "wv": np.ascontiguousarray(np.asarray(Wv, dtype=np.float32)),
        "wo": np.ascontiguousarray(np.asarray(Wo, dtype=np.float32)),
        "cos": cosf, "sin": sinf,
        "ident": np.eye(128, dtype=np.float32),
        "masks": maskdata,
        "wqn": np.ascontiguousarray(
            np.tile(np.asarray(q_norm_w, np.float32), 2).reshape(128, 1)),
        "wkn": np.ascontiguousarray(
            np.tile(np.asarray(k_norm_w, np.float32), 2).reshape(128, 1)),
    }
    in_maps = []
    for c in range(NCORES):
        m = dict(shared)
        m["x"] = np.ascontiguousarray(x[c * BL:(c + 1) * BL].reshape(BL * T, C))
        in_maps.append(m)
    return in_maps


def kernel(x, cos, sin, attn_mask, Wq, Wk, Wv, Wo, q_norm_w, k_norm_w):
    nc, tiles = _prepare(attn_mask, q_norm_w, k_norm_w)
    in_maps = make_in_maps(x, cos, sin, attn_mask, Wq, Wk, Wv, Wo,
                           q_norm_w, k_norm_w, tiles)
    res = run_bass_kernel_spmd(nc, in_maps, core_ids=list(range(NCORES)))
    out = np.concatenate([res.results[c]["y"].reshape(BL, T, C)
                          for c in range(NCORES)], axis=0)
    return out


# revision 10
# speedup vs baseline: 1.1624x; 1.1624x over previous
"""Trainium2 Bass kernel for GQA multi-head attention with RoPE + QK-RMSNorm
and a block-staircase mask (block-diffusion attention).

Strategy: data-parallel over batch across 8 NeuronCores (2 batches/core).
Per core, fully fused pipeline in fp32r (TF32-like) matmuls:
  x -> xT (PE transpose) -> q/k/v projections -> RoPE+RMSNorm (DVE/ACT)
  -> qT/kT via PE transpose (head-paired layout, halves of 128 partitions)
  -> scores S^T = kT.T @ qT per kv-group (K=64 matmuls, both halves)
  -> exp (ACT, fused 1/sqrt(d) scale) -> 0/1 mask multiply (DVE)
  -> AV: yT = [v|1].T @ E^T accumulated over key tiles (denominator fused
     as an extra ones column -> Z row) -> ones-matmul partition broadcast
     of Z -> reciprocal -> divide -> Wo projection -> y.

The attention mask is analyzed host-side per 128x128 block (full / zero /
partial); zero blocks are skipped, partial blocks get a 0/1 multiply.
"""
import sys

sys.path.insert(0, "/opt/trn_rl_repo")

from contextlib import ExitStack

import numpy as np

import concourse.bass as bass
import concourse.tile as tile
from concourse import bacc, mybir
from concourse.bass_utils import run_bass_kernel_spmd

F32 = mybir.dt.float32
F32R = mybir.dt.float32r
ACT = mybir.ActivationFunctionType
AX = mybir.AxisListType

B, T, C = 16, 1024, 1024
H, G, D = 16, 4, 64          # query heads, kv heads, head dim
NCORES = 8
BL = B // NCORES             # batches per core
TT = T // 128                # 8 row tiles
QCN = 4                      # 256-wide query chunks per sequence
EPS = 1e-6
SCALE = 1.0 / np.sqrt(D)

# head-pair permutation: transpose pair j holds (PI[2j] -> half0, PI[2j+1] -> half1)
# so kv-group g lives in half g%2, slots (g//2)*4..+3, head = 4g + slot-offset
PI = []
for _j in range(8):
    _a, _b = (_j, _j + 4) if _j < 4 else (_j + 4, _j + 8)
    PI += [_a, _b]


def _analyze_mask(attn_mask):
    """Per 128x128 block classification in S^T orientation."""
    classes = [[None] * TT for _ in range(TT)]
    tiles = []
    for qt in range(TT):
        for kt in range(TT):
            blk = attn_mask[qt * 128:(qt + 1) * 128, kt * 128:(kt + 1) * 128]
            if blk.all():
                classes[qt][kt] = "full"
            elif not blk.any():
                classes[qt][kt] = "zero"
            else:
                mt = blk.T.astype(np.float32)  # S^T orientation: [key, query]
                for i, ex in enumerate(tiles):
                    if np.array_equal(ex, mt):
                        classes[qt][kt] = i
                        break
                else:
                    tiles.append(mt)
                    classes[qt][kt] = len(tiles) - 1
    return classes, tiles


def build_program(classes, n_masks, use_qw, use_kw):
    nc = bacc.Bacc("TRN2", target_bir_lowering=False, debug=False)

    x_d = nc.dram_tensor("x", [BL * T, C], F32, kind="ExternalInput")
    wq_d = nc.dram_tensor("wq", [C, H * D], F32, kind="ExternalInput")
    wk_d = nc.dram_tensor("wk", [C, G * D], F32, kind="ExternalInput")
    wv_d = nc.dram_tensor("wv", [C, G * D], F32, kind="ExternalInput")
    wo_d = nc.dram_tensor("wo", [H * D, C], F32, kind="ExternalInput")
    cos_d = nc.dram_tensor("cos", [T, D // 2], F32, kind="ExternalInput")
    sin_d = nc.dram_tensor("sin", [T, D // 2], F32, kind="ExternalInput")
    id_d = nc.dram_tensor("ident", [128, 128], F32, kind="ExternalInput")
    nm = max(1, n_masks)
    mask_d = nc.dram_tensor("masks", [nm, 128, 128], F32, kind="ExternalInput")
    wqn_d = nc.dram_tensor("wqn", [128, 1], F32, kind="ExternalInput")
    wkn_d = nc.dram_tensor("wkn", [128, 1], F32, kind="ExternalInput")
    y_d = nc.dram_tensor("y", [BL * T, C], F32, kind="ExternalOutput")

    with tile.TileContext(nc) as tc, ExitStack() as st:
        const = st.enter_context(tc.tile_pool(name="const", bufs=1))
        scr = st.enter_context(tc.tile_pool(name="scr", bufs=2))
        bat = st.enter_context(tc.tile_pool(name="bat", bufs=1))
        xnp = st.enter_context(tc.tile_pool(name="xnp", bufs=2))
        ropep = st.enter_context(tc.tile_pool(name="ropep", bufs=3))
        small = st.enter_context(tc.tile_pool(name="small", bufs=2))
        qtp = st.enter_context(tc.tile_pool(name="qtp", bufs=1))
        erp = st.enter_context(tc.tile_pool(name="erp", bufs=2))
        yp = st.enter_context(tc.tile_pool(name="yp", bufs=1))
        youtp = st.enter_context(tc.tile_pool(name="youtp", bufs=2))
        knp = st.enter_context(tc.tile_pool(name="knp", bufs=1))

        pp = st.enter_context(tc.tile_pool(name="pp", bufs=2, space="PSUM"))
        ptp = st.enter_context(tc.tile_pool(name="ptp", bufs=2, space="PSUM"))
        psp = st.enter_context(tc.tile_pool(name="psp", bufs=2, space="PSUM"))
        pyp = st.enter_context(tc.tile_pool(name="pyp", bufs=1, space="PSUM"))

        # ---------------- constants ----------------
        id_sb = const.tile([128, 128], F32, tag="id")
        nc.sync.dma_start(id_sb[:], id_d[:])
        cos_sb = const.tile([128, TT, 32], F32, tag="cos")
        sin_sb = const.tile([128, TT, 32], F32, tag="sin")
        nc.sync.dma_start(cos_sb[:], cos_d.ap().rearrange("(tt p) d -> p tt d", p=128))
        nc.sync.dma_start(sin_sb[:], sin_d.ap().rearrange("(tt p) d -> p tt d", p=128))

        maskr = const.tile([128, nm, 128], F32R, tag="maskr")
        mf = scr.tile([128, nm, 128], F32, tag="wchunk")
        nc.sync.dma_start(mf[:], mask_d.ap().rearrange("n p k -> p n k"))
        nc.scalar.activation(maskr[:], mf[:], ACT.Copy)

        onesr = const.tile([128, 64], F32R, tag="onesr")
        ones_f = scr.tile([128, 64], F32, tag="wchunk")
        nc.vector.memset(ones_f[:], 1.0)
        nc.scalar.activation(onesr[64:65, :], ones_f[64:65, :], ACT.Copy)

        w2q = const.tile([128, 1], F32, tag="w2q")
        w2k = const.tile([128, 1], F32, tag="w2k")
        nc.sync.dma_start(w2q[:], wqn_d[:])
        nc.sync.dma_start(w2k[:], wkn_d[:])
        epsb = const.tile([128, 1], F32, tag="epsb")
        nc.vector.memset(epsb[:], EPS)

        # weights, rounded to f32r through a staging chunk
        wqr = const.tile([128, 8, H * D], F32R, tag="wqr")
        wkr = const.tile([128, 8, G * D], F32R, tag="wkr")
        wvr = const.tile([128, 8, G * D], F32R, tag="wvr")
        wor = const.tile([128, 8, C], F32R, tag="wor")
        for w_d, w_r, m in ((wq_d, wqr, H * D), (wk_d, wkr, G * D),
                            (wv_d, wvr, G * D), (wo_d, wor, C)):
            wv_ap = w_d.ap().rearrange("(ct p) m -> p ct m", p=128)
            for ct in range(8):
                chunk = scr.tile([128, 1024], F32, tag="wchunk")
                nc.sync.dma_start(chunk[:, 0:m], wv_ap[:, ct, :])
                nc.scalar.activation(w_r[:, ct, :], chunk[:, 0:m], ACT.Copy)

        # ---------------- per batch ----------------
        for b in range(BL):
            xTr = bat.tile([128, 8, T], F32R, tag="xTr")     # [c_part, ct, t]
            kTr = bat.tile([128, 2, T], F32R, tag="kTr")     # [d(2 kv), pair, t]
            vr = bat.tile([128, TT, G, 65], F32R, tag="vr")  # [key, tt, g, d+1]

            # ones column of v_ext
            of = scr.tile([128, 32], F32, tag="wchunk")
            nc.vector.memset(of[:], 1.0)
            nc.vector.tensor_copy(
                vr[:, :, :, 64:65],
                of[:].rearrange("p (a b c) -> p a b c", a=TT, b=G))

            # --- xT ---
            for tt in range(TT):
                xn = xnp.tile([128, C], F32, tag="xn")
                nc.sync.dma_start(
                    xn[:], x_d[b * T + tt * 128: b * T + (tt + 1) * 128, :])
                for ct in range(8):
                    ptile = ptp.tile([128, 128], F32, tag="pt")
                    nc.tensor.transpose(ptile[:], xn[:, ct * 128:(ct + 1) * 128],
                                        id_sb[:])
                    nc.vector.tensor_copy(xTr[:, ct, tt * 128:(tt + 1) * 128],
                                          ptile[:])

            # --- k/v projections + rope/norm + transpose ---
            for tt in range(TT):
                pk = pp.tile([128, 512], F32, tag="pp")
                for ct in range(8):
                    nc.tensor.matmul(pk[:, 0:G * D],
                                     xTr[:, ct, tt * 128:(tt + 1) * 128],
                                     wkr[:, ct, :], start=(ct == 0), stop=(ct == 7))
                pk4 = pk[:, 0:G * D].rearrange("p (g h d) -> p g h d", g=G, h=2)
                cosb = cos_sb[:, tt, :].unsqueeze(1).unsqueeze(1).to_broadcast(
                    (128, G, 2, 32))
                sinb = sin_sb[:, tt, :].unsqueeze(1).unsqueeze(1).to_broadcast(
                    (128, G, 2, 32))
                ka = ropep.tile([128, 1024], F32, tag="rope")
                kb = ropep.tile([128, 1024], F32, tag="rope")
                ka4 = ka[:, 0:G * D].rearrange("p (g h d) -> p g h d", g=G, h=2)
                kb4 = kb[:, 0:G * D].rearrange("p (g h d) -> p g h d", g=G, h=2)
                nc.vector.tensor_mul(ka4, pk4, cosb)
                nc.vector.tensor_mul(kb4, pk4, sinb)
                kn = knp.tile([128, G * D], F32, tag="knorm")
                kn4 = kn[:].rearrange("p (g h d) -> p g h d", g=G, h=2)
                nc.vector.tensor_add(kn4[:, :, 0, :], ka4[:, :, 0, :], kb4[:, :, 1, :])
                nc.vector.tensor_sub(kn4[:, :, 1, :], ka4[:, :, 1, :], kb4[:, :, 0, :])
                sq = ropep.tile([128, 1024], F32, tag="rope")
                nc.vector.tensor_mul(sq[:, 0:G * D], kn[:], kn[:])
                ss = small.tile([128, G], F32, tag="ss")
                nc.vector.reduce_sum(
                    ss[:], sq[:, 0:G * D].rearrange("p (g d) -> p g d", g=G),
                    axis=AX.X)
                srt = small.tile([128, G], F32, tag="srt")
                nc.scalar.activation(srt[:], ss[:], ACT.Sqrt, bias=epsb[:], scale=1.0 / D)
                sinv = small.tile([128, G], F32, tag="sinv")
                nc.vector.reciprocal(sinv[:], srt[:])
                nc.vector.tensor_mul(
                    kn[:].rearrange("p (g d) -> p g d", g=G),
                    kn[:].rearrange("p (g d) -> p g d", g=G),
                    sinv[:].unsqueeze(2).to_broadcast((128, G, D)))
                for j in range(2):
                    ptile = ptp.tile([128, 128], F32, tag="pt")
                    nc.tensor.transpose(ptile[:], kn[:, j * 128:(j + 1) * 128],
                                        id_sb[:])
                    kw = {"scale": w2k[:]} if use_kw else {}
                    nc.scalar.activation(kTr[:, j, tt * 128:(tt + 1) * 128],
                                         ptile[:], ACT.Copy, **kw)
                pv = pp.tile([128, 512], F32, tag="pp")
                for ct in range(8):
                    nc.tensor.matmul(pv[:, 0:G * D],
                                     xTr[:, ct, tt * 128:(tt + 1) * 128],
                                     wvr[:, ct, :], start=(ct == 0), stop=(ct == 7))
                nc.scalar.activation(
                    vr[:, tt, :, 0:64],
                    pv[:, 0:G * D].rearrange("p (g d) -> p g d", g=G), ACT.Copy)

            # --- query chunks ---
            for qc in range(QCN):
                qTr = qtp.tile([128, 8, 256], F32R, tag="qTr")  # [d, slot, tq]
                for tl in range(2):
                    tt = 2 * qc + tl
                    qn = ropep.tile([128, 1024], F32, tag="qn", bufs=2)
                    for mh in range(2):
                        pq = pp.tile([128, 512], F32, tag="pp")
                        for ct in range(8):
                            nc.tensor.matmul(
                                pq[:], xTr[:, ct, tt * 128:(tt + 1) * 128],
                                wqr[:, ct, mh * 512:(mh + 1) * 512],
                                start=(ct == 0), stop=(ct == 7))
                        pq4 = pq[:].rearrange("p (g h d) -> p g h d", g=8, h=2)
                        cosb = cos_sb[:, tt, :].unsqueeze(1).unsqueeze(1).to_broadcast(
                            (128, 8, 2, 32))
                        sinb = sin_sb[:, tt, :].unsqueeze(1).unsqueeze(1).to_broadcast(
                            (128, 8, 2, 32))
                        qa = ropep.tile([128, 1024], F32, tag="rope")
                        qb = ropep.tile([128, 1024], F32, tag="rope")
                        qa4 = qa[:, 0:512].rearrange("p (g h d) -> p g h d", g=8, h=2)
                        qb4 = qb[:, 0:512].rearrange("p (g h d) -> p g h d", g=8, h=2)
                        nc.vector.tensor_mul(qa4, pq4, cosb)
                        nc.vector.tensor_mul(qb4, pq4, sinb)
                        qn4 = qn[:, mh * 512:(mh + 1) * 512].rearrange(
                            "p (g h d) -> p g h d", g=8, h=2)
                        nc.vector.tensor_add(qn4[:, :, 0, :], qa4[:, :, 0, :],
                                             qb4[:, :, 1, :])
                        nc.vector.tensor_sub(qn4[:, :, 1, :], qa4[:, :, 1, :],
                                             qb4[:, :, 0, :])
                        sq = ropep.tile([128, 1024], F32, tag="rope")
                        nc.vector.tensor_mul(sq[:, 0:512],
                                             qn[:, mh * 512:(mh + 1) * 512],
                                             qn[:, mh * 512:(mh + 1) * 512])
                        ss = small.tile([128, 8], F32, tag="ss")
                        nc.vector.reduce_sum(
                            ss[:], sq[:, 0:512].rearrange("p (g d) -> p g d", g=8),
                            axis=AX.X)
                        srt = small.tile([128, 8], F32, tag="srt")
                        nc.scalar.activation(srt[:], ss[:], ACT.Sqrt, bias=epsb[:],
                                             scale=1.0 / D)
                        sinv = small.tile([128, 8], F32, tag="sinv")
                        nc.vector.reciprocal(sinv[:], srt[:])
                        nc.vector.tensor_mul(
                            qn[:, mh * 512:(mh + 1) * 512].rearrange(
                                "p (g d) -> p g d", g=8),
                            qn[:, mh * 512:(mh + 1) * 512].rearrange(
                                "p (g d) -> p g d", g=8),
                            sinv[:].unsqueeze(2).to_broadcast((128, 8, D)))
                        for j in range(4 * mh, 4 * mh + 4):
                            ptile = ptp.tile([128, 128], F32, tag="pt")
                            nc.tensor.transpose(ptile[:],
                                                qn[:, j * 128:(j + 1) * 128],
                                                id_sb[:])
                            kw = {"scale": w2q[:]} if use_qw else {}
                            nc.scalar.activation(
                                qTr[:, j, tl * 128:(tl + 1) * 128],
                                ptile[:], ACT.Copy, **kw)


                # --- attention for this chunk ---
                yup = yp.tile([128, 8, 256], F32, tag="yup")
                yTwo = yp.tile([128, 8, 256], F32R, tag="yTwo")

                kjs = []
                for kj in range(TT):
                    cls = [classes[2 * qc + tl][kj] for tl in range(2)]
                    if cls[0] == "zero" and cls[1] == "zero":
                        continue
                    kjs.append((kj, cls))

                for g in range(G):
                    u = g % 2
                    sb0 = (g // 2) * 4
                    py = pyp.tile([65, 4, 256], F32, tag="py")
                    for idx, (kj, cls) in enumerate(kjs):
                        er = erp.tile([128, 4, 256], F32R, tag="er")
                        for sp in range(2):
                            ps = psp.tile([128, 512], F32, tag="ps")
                            nc.tensor.matmul(
                                ps[:],
                                kTr[u * 64:(u + 1) * 64, g // 2,
                                    kj * 128:(kj + 1) * 128],
                                qTr[u * 64:(u + 1) * 64,
                                    sb0 + 2 * sp: sb0 + 2 * sp + 2, :],
                                tile_position=(u * 64, 0))
                            nc.scalar.activation(
                                er[:, 2 * sp:2 * sp + 2, :].rearrange(
                                    "p a b -> p (a b)"),
                                ps[:], ACT.Exp, scale=float(SCALE))
                        for tl in range(2):
                            c = cls[tl]
                            if c == "full":
                                continue
                            reg = er[:, :, tl * 128:(tl + 1) * 128]
                            if c == "zero":
                                nc.vector.tensor_scalar_mul(reg, reg, 0.0)
                            else:
                                nc.vector.tensor_mul(
                                    reg, reg,
                                    maskr[:, c:c + 1, :].to_broadcast((128, 4, 128)))
                        last = idx == len(kjs) - 1
                        for jp in range(2):
                            nc.tensor.matmul(py[:, 2 * jp:2 * jp + 2, :],
                                             vr[:, kj, g, :],
                                             er[:, 2 * jp:2 * jp + 2, :],
                                             start=(idx == 0), stop=last)
                    # denominator: Z row -> broadcast -> reciprocal -> divide
                    zr = ropep.tile([65, 1024], F32R, tag="rope")
                    nc.scalar.activation(
                        zr[64:65, :],
                        py[64:65, :, :].rearrange("p a b -> p (a b)"), ACT.Copy)
                    zinv = ropep.tile([64, 1024], F32, tag="rope")
                    zscr = ropep.tile([64, 1024], F32, tag="rope")
                    for zh in range(2):
                        pz = psp.tile([64, 512], F32, tag="ps")
                        nc.tensor.matmul(pz[:], onesr[64:65, :],
                                         zr[64:65, zh * 512:(zh + 1) * 512],
                                         tile_position=(64, 0))
                        nc.vector.reciprocal_approx_accurate(
                            zinv[:, zh * 512:(zh + 1) * 512], pz[:],
                            zscr[:, zh * 512:(zh + 1) * 512])
                    for j in range(4):
                        h = 4 * g + j
                        p = h // 2
                        if h % 2 == 0:
                            nc.vector.tensor_mul(yTwo[0:64, p, :], py[0:64, j, :],
                                                 zinv[:, j * 256:(j + 1) * 256])
                        else:
                            nc.vector.tensor_mul(yup[0:64, p, :], py[0:64, j, :],
                                                 zinv[:, j * 256:(j + 1) * 256])
                    # shift this group's odd heads up and round to f32r
                    nc.sync.dma_start(yup[64:128, 2 * g:2 * g + 2, :],
                                      yup[0:64, 2 * g:2 * g + 2, :])
                    nc.scalar.activation(yTwo[64:128, 2 * g:2 * g + 2, :],
                                         yup[64:128, 2 * g:2 * g + 2, :], ACT.Copy)


                # --- Wo projection ---
                for tl in range(2):
                    for nh in range(2):
                        po = psp.tile([128, 512], F32, tag="ps")
                        for mt in range(8):
                            nc.tensor.matmul(
                                po[:], yTwo[:, mt, tl * 128:(tl + 1) * 128],
                                wor[:, mt, nh * 512:(nh + 1) * 512],
                                start=(mt == 0), stop=(mt == 7))
                        yout = youtp.tile([128, 512], F32, tag="yout")
                        nc.scalar.activation(yout[:], po[:], ACT.Copy)
                        r0 = b * T + qc * 256 + tl * 128
                        nc.sync.dma_start(
                            y_d[r0:r0 + 128, nh * 512:(nh + 1) * 512], yout[:])

    nc.compile()
    return nc


_CACHE = {}


def _prepare(attn_mask, q_norm_w, k_norm_w):
    classes, tiles = _analyze_mask(np.asarray(attn_mask, dtype=bool))
    use_qw = not np.allclose(np.asarray(q_norm_w), 1.0)
    use_kw = not np.allclose(np.asarray(k_norm_w), 1.0)
    key = (tuple(tuple(r) for r in classes), len(tiles), use_qw, use_kw)
    if key not in _CACHE:
        _CACHE[key] = (build_program(classes, len(tiles), use_qw, use_kw), tiles)
    return _CACHE[key]


def make_in_maps(x, cos, sin, attn_mask, Wq, Wk, Wv, Wo, q_norm_w, k_norm_w, tiles):
    x = np.ascontiguousarray(np.asarray(x, dtype=np.float32))
    cosf = np.ascontiguousarray(np.asarray(cos, np.float32).reshape(T, D // 2))
    sinf = np.ascontiguousarray(np.asarray(sin, np.float32).reshape(T, D // 2))

    wq = np.asarray(Wq, dtype=np.float32)
    wq_p = np.empty_like(wq)
    for j in range(8):
        a, b2 = PI[2 * j], PI[2 * j + 1]
        wq_p[:, 128 * j:128 * j + 64] = wq[:, 64 * a:64 * a + 64]
        wq_p[:, 128 * j + 64:128 * j + 128] = wq[:, 64 * b2:64 * b2 + 64]

    nmask = max(1, len(tiles))
    maskdata = np.zeros((nmask, 128, 128), dtype=np.float32)
    for i, t in enumerate(tiles):
        maskdata[i] = t

    shared = {
        "wq": np.ascontiguousarray(wq_p),
        "wk": np.ascontiguousarray(np.asarray(Wk, dtype=np.float32)),
        "wv": np.ascontiguousarray(np.asarray(Wv, dtype=np.float32)),
        "wo": np.ascontiguousarray(np.asarray(Wo, dtype=np.float32)),
        "cos": cosf, "sin": sinf,
        "ident": np.eye(128, dtype=np.float32),
        "masks": maskdata,
        "wqn": np.ascontiguousarray(
            np.tile(np.asarray(q_norm_w, np.float32), 2).reshape(128, 1)),
        "wkn": np.ascontiguousarray(
            np.tile(np.asarray(k_norm_w, np.float32), 2).reshape(128, 1)),
    }
    in_maps = []
    for c in range(NCORES):
        m = dict(shared)
        m["x"] = np.ascontiguousarray(x[c * BL:(c + 1) * BL].reshape(BL * T, C))
        in_maps.append(m)
    return in_maps


def kernel(x, cos, sin, attn_mask, Wq, Wk, Wv, Wo, q_norm_w, k_norm_w):
    nc, tiles = _prepare(attn_mask, q_norm_w, k_norm_w)
    in_maps = make_in_maps(x, cos, sin, attn_mask, Wq, Wk, Wv, Wo,
                           q_norm_w, k_norm_w, tiles)
    res = run_bass_kernel_spmd(nc, in_maps, core_ids=list(range(NCORES)))
    out = np.concatenate([res.results[c]["y"].reshape(BL, T, C)
                          for c in range(NCORES)], axis=0)
    return out
